# revision 1
# baseline (speedup 1.0000x reference)
"""Multi-head graph attention layer (GAT) for Trainium2, 8-core data-parallel.

Problem: B=8, N=1024, D_IN=256, D_OUT=64, H=8, LeakyReLU slope 0.2.
  Wh = einsum('bnd,hdf->bhnf', h, W)
  f1 = Wh @ a1, f2 = Wh @ a2              (per head)
  e  = leaky_relu(f1[:,None] + f2[None,:])
  att = softmax(where(adj==0, -inf, e))
  out = att @ Wh  -> concat heads [B, N, H*F]

Sharding: one batch element per NeuronCore (B=8 across 8 cores).

Core algebraic trick: with x = f1_i + f2_j,
  exp(leaky_relu(x)) = max(exp(x), exp(0.2 x))
                     = max(E1_i*E2_j, E1s_i*E2s_j)
(exponentials of rank-1 scores), so the [N,N] attention matrix needs no
per-element transcendentals: only per-row/col exp vectors + elementwise
mult/max/mask — all DVE/ACT/GPSIMD-class ops. Unnormalized attention is
accumulated against [Wh | 1] so column 64 of the output matmul yields the
softmax denominator Z; normalization happens after a PE transpose.

All [N,N]-scale work is done in a transposed "U[j,i]" layout so that U can
be streamed directly as the moving operand of out^T = [Wh|1]^T @ U.
"""

import numpy as np
import ml_dtypes

BF16 = ml_dtypes.bfloat16

B, N, D_IN, D_OUT, H = 8, 1024, 256, 64, 8
NEG_SLOPE = 0.2
P = 128                       # partitions
NJT = N // P                  # 8 j-tiles
NIT = N // P                  # 8 i-tiles
NKT = D_IN // P               # 2 k-tiles
HF = H * D_OUT                # 512
AUG = D_OUT + 1               # 65 (Wh columns + ones column)

# Engine balancing knobs:
GPSIMD_JTS = (1, 3, 5, 6)     # j-tiles whose mask op runs on gpsimd (of 8)
DVE_T_JTS = (7,)              # j-tiles whose T-build runs on DVE (rest: ACT)

_F32R = True                  # use float32r for the Wh / f matmuls


def _build_program():
    """Build the single-core SPMD Bass program. Returns (nc, names)."""
    import concourse.bass as bass
    import concourse.bacc as bacc
    import concourse.tile as tile
    from concourse import mybir
    from concourse.masks import make_identity

    f32 = mybir.dt.float32
    bf16 = mybir.dt.bfloat16
    AF = mybir.ActivationFunctionType
    OP = mybir.AluOpType

    nc = bacc.Bacc("TRN2", target_bir_lowering=False, debug=False,
                   enable_asserts=False, num_devices=8)

    mmdt = mybir.dt.float32r if _F32R else f32
    hT = nc.dram_tensor("hT", [D_IN, N], mmdt, kind="ExternalInput").ap()
    adjT = nc.dram_tensor("adjT", [N, N], bf16, kind="ExternalInput").ap()
    wrs = nc.dram_tensor("wrs", [D_IN, HF], mmdt, kind="ExternalInput").ap()
    w12 = nc.dram_tensor("w12", [D_IN, 2 * H], mmdt,
                         kind="ExternalInput").ap()
    out = nc.dram_tensor("out", [N, HF], f32, kind="ExternalOutput").ap()

    with tile.TileContext(nc) as tc:
        with (
            tc.tile_pool(name="const", bufs=1) as const,
            tc.tile_pool(name="inputs", bufs=1) as inputs,
            tc.tile_pool(name="whp", bufs=1) as whp,
            tc.tile_pool(name="ecol", bufs=1) as ecolp,
            tc.tile_pool(name="ps_s", bufs=3, space="PSUM") as ps_s,
            tc.tile_pool(name="ps_ot", bufs=2, space="PSUM") as ps_ot,
            tc.tile_pool(name="bcast", bufs=2) as bcastp,
            tc.tile_pool(name="work", bufs=3) as work,
            tc.tile_pool(name="fin", bufs=3) as fin,
            tc.tile_pool(name="dram", bufs=1, space="DRAM") as dramp,
        ):
            # ---- Phase 0: load inputs -------------------------------------
            ident = const.tile([P, P], f32)
            make_identity(nc, ident)

            ht_sb = []
            for kt in range(NKT):
                t = inputs.tile([P, N], mmdt, tag=f"ht{kt}")
                nc.sync.dma_start(out=t, in_=hT[kt * P:(kt + 1) * P, :])
                ht_sb.append(t)
            wrs_sb = []
            for kt in range(NKT):
                t = inputs.tile([P, HF], mmdt, tag=f"wrs{kt}")
                nc.sync.dma_start(out=t, in_=wrs[kt * P:(kt + 1) * P, :])
                wrs_sb.append(t)
            w12_sb = []
            for kt in range(NKT):
                t = inputs.tile([P, 2 * H], mmdt, tag=f"w12{kt}")
                nc.sync.dma_start(out=t, in_=w12[kt * P:(kt + 1) * P, :])
                w12_sb.append(t)
            adj_sb = []
            for jt in range(NJT):
                t = inputs.tile([P, N], bf16, tag=f"adj{jt}")
                nc.sync.dma_start(out=t, in_=adjT[jt * P:(jt + 1) * P, :])
                adj_sb.append(t)

            # ---- Phase 1: f vectors and their exponentials ----------------
            # f_ps[it] = [128, 16] scores: cols 0-7 f1 per head, 8-15 f2.
            # ecols[it][:, h]     = exp(0.2 f2_h)  (E2s - whaug scale + Z col)
            # ecols[it][:, 8 + h] = exp(0.8 f2_h)  (c - the T-build scale)
            # e12t rows 0-7: exp(f1_h); rows 32-39: exp(0.2 f1_h)
            # (section 2 starts at partition 32 - engine base-partition rule)
            e12t = const.tile([40, N], bf16)
            ecols = []
            for it in range(NIT):
                ps = ps_s.tile([P, 2 * H], f32, tag='pss')
                for kt in range(NKT):
                    lhsT = ht_sb[kt][:, it * P:(it + 1) * P]
                    rhs = w12_sb[kt]
                    nc.tensor.matmul(ps, lhsT, rhs,
                                     start=(kt == 0), stop=(kt == NKT - 1))
                ec = ecolp.tile([P, 2 * H], f32, tag=f"ecols{it}")
                nc.scalar.activation(ec[:, 0:H], ps[:, H:2 * H], AF.Exp,
                                     scale=NEG_SLOPE)
                nc.scalar.activation(ec[:, H:2 * H], ps[:, H:2 * H], AF.Exp,
                                     scale=1.0 - NEG_SLOPE)
                ecols.append(ec)
                # transpose f tile -> [16, 128], then exp into e12t columns
                fsb = work.tile([P, 2 * H], f32, tag="fsb")
                nc.vector.tensor_copy(fsb, ps)
                tr = ps_s.tile([2 * H, P], f32, tag='pss')
                nc.tensor.transpose(tr, fsb, ident)
                nc.scalar.activation(e12t[0:H, it * P:(it + 1) * P],
                                     tr[0:H, :], AF.Exp, scale=1.0)
                nc.scalar.activation(e12t[32:32 + H, it * P:(it + 1) * P],
                                     tr[0:H, :], AF.Exp, scale=NEG_SLOPE)

            # ---- Phase 2: Wh = h @ W, scaled by E2s, augmented Z col ------
            # whaug[jt][:, h, 0:64] = Wh_h[j,:] * exp(0.2 f2_h[j])
            # whaug[jt][:, h, 64]   = exp(0.2 f2_h[j])
            # (the E2s_j factor of U is folded into the matmul weights)
            whaug = []
            for it in range(NIT):
                ps = ps_s.tile([P, HF], f32, tag='pss')
                for kt in range(NKT):
                    lhsT = ht_sb[kt][:, it * P:(it + 1) * P]
                    rhs = wrs_sb[kt]
                    nc.tensor.matmul(ps, lhsT, rhs,
                                     start=(kt == 0), stop=(kt == NKT - 1))
                wa = whp.tile([P, H, AUG], bf16, tag=f"whaug{it}")
                for h in range(H):
                    nc.vector.tensor_scalar_mul(
                        wa[:, h, 0:D_OUT],
                        ps[:, h * D_OUT:(h + 1) * D_OUT],
                        ecols[it][:, h:h + 1])
                nc.vector.tensor_copy(wa[:, :, D_OUT], ecols[it][:, 0:H])
                whaug.append(wa)

            # output accumulators: one [128, 512] f32 tile per i-tile,
            # written head-by-head, DMA'd once at the end (batched output)
            out_sb = []
            for it in range(NIT):
                osb = whp.tile([P, HF], f32, tag=f"osb{it}")
                out_sb.append(osb)

            # bounce e12t through DRAM so broadcasts can use 0-stride reads
            e12t_dram = dramp.tile([40, N], bf16)
            nc.sync.dma_start(out=e12t_dram[0:H, :], in_=e12t[0:H, :])
            nc.sync.dma_start(out=e12t_dram[32:32 + H, :],
                              in_=e12t[32:32 + H, :])

            # ---- Phase 3: per-head attention ------------------------------
            for h in range(H):
                e1bc = bcastp.tile([P, N], bf16, tag="e1bc")
                nc.sync.dma_start(
                    out=e1bc,
                    in_=e12t_dram[h:h + 1, :].partition_broadcast(P))
                e1sbc = bcastp.tile([P, N], bf16, tag="e1sbc")
                nc.sync.dma_start(
                    out=e1sbc,
                    in_=e12t_dram[32 + h:32 + h + 1, :]
                        .partition_broadcast(P))

                ot = ps_ot.tile([AUG, N], f32, tag="ot")
                for jt in range(NJT):
                    e2scol = ecols[jt][:, h:h + 1]          # exp(0.2 f2_h)
                    ccol = ecols[jt][:, H + h:H + h + 1]    # exp(0.8 f2_h)
                    # T = E1 * c_j   (then U = E2s_j*max(T, E1s)*m, with the
                    # E2s_j factor folded into whaug)
                    pt = work.tile([P, N], bf16, tag="pt")
                    if jt in DVE_T_JTS:
                        nc.vector.tensor_scalar_mul(pt, e1bc, ccol)
                    else:
                        nc.scalar.activation(pt, e1bc, AF.Copy, scale=ccol)
                    # V = max(T, E1s)
                    ut = work.tile([P, N], bf16, tag="ut")
                    nc.vector.tensor_tensor(out=ut, in0=pt, in1=e1sbc,
                                            op=OP.max)
                    # U' = V * mask
                    um = work.tile([P, N], bf16, tag="um")
                    eng = nc.gpsimd if jt in GPSIMD_JTS else nc.vector
                    eng.tensor_tensor(out=um, in0=ut, in1=adj_sb[jt],
                                      op=OP.mult)
                    # out^T[(f|1), i] += [Wh*E2s | E2s]^T @ U'
                    lhsT = whaug[jt][:, h, :]
                    for nh in range(2):
                        nc.tensor.matmul(
                            ot[:, nh * 512:(nh + 1) * 512], lhsT,
                            um[:, nh * 512:(nh + 1) * 512],
                            start=(jt == 0), stop=(jt == NJT - 1))

                # finalize: evacuate, transpose, normalize by Z, store
                ots = fin.tile([AUG, N], f32, tag="ots")
                nc.scalar.copy(ots, ot)
                for it in range(NIT):
                    tr2 = ps_s.tile([P, AUG], f32, tag='pss')
                    nc.tensor.transpose(tr2, ots[:, it * P:(it + 1) * P],
                                        ident[0:AUG, 0:AUG])
                    rcol = fin.tile([P, 1], f32, tag="rcol")
                    nc.vector.reciprocal(rcol, tr2[:, D_OUT:AUG])
                    nc.vector.tensor_scalar_mul(
                        out_sb[it][:, h * D_OUT:(h + 1) * D_OUT],
                        tr2[:, 0:D_OUT], rcol)

            for it in range(NIT):
                nc.sync.dma_start(out=out[it * P:(it + 1) * P, :],
                                  in_=out_sb[it])

    nc.compile()
    return nc


def _host_prep(h, adj, W, a):
    """Host-side input prep: transposes / casts / tiny einsums only."""
    a1, a2 = a[:, :D_OUT], a[:, D_OUT:]
    w1 = np.einsum("hdf,hf->hd", W, a1).astype(np.float32)   # [H, D_IN]
    w2 = np.einsum("hdf,hf->hd", W, a2).astype(np.float32)
    w12 = np.concatenate([w1.T, w2.T], axis=1)               # [D_IN, 16]
    wrs = np.ascontiguousarray(
        W.transpose(1, 0, 2).reshape(D_IN, HF)).astype(np.float32)
    in_maps = []
    for b in range(B):
        in_maps.append({
            "hT": np.ascontiguousarray(h[b].T).astype(np.float32),
            "adjT": np.ascontiguousarray(adj[b].T).astype(BF16),
            "wrs": wrs,
            "w12": w12,
        })
    return in_maps


def kernel(h, adj, W, a):
    from concourse.bass_utils import run_bass_kernel_spmd

    in_maps = _host_prep(np.asarray(h), np.asarray(adj),
                         np.asarray(W), np.asarray(a))
    nc = _build_program()
    res = run_bass_kernel_spmd(nc, in_maps, core_ids=list(range(B)))
    out = np.stack([np.asarray(res.results[b]["out"]) for b in range(B)])
    return out.astype(np.float32)



# revision 9
# speedup vs baseline: 1.5228x; 1.5228x over previous
"""Multi-head graph attention layer (GAT) for Trainium2, 8-core data-parallel.

Problem: B=8, N=1024, D_IN=256, D_OUT=64, H=8, LeakyReLU slope 0.2.
  Wh = einsum('bnd,hdf->bhnf', h, W)
  f1 = Wh @ a1, f2 = Wh @ a2              (per head)
  e  = leaky_relu(f1[:,None] + f2[None,:])
  att = softmax(where(adj==0, -inf, e))
  out = att @ Wh  -> concat heads [B, N, H*F]

Sharding: one batch element per NeuronCore (B=8 across 8 cores).

Algebra: with x = f1_i + f2_j,
  exp(leaky_relu(x)) = max(exp(x), exp(0.2 x))
                     = E1s_i * E2s_j * max(d_i * c_j, 1)
where d = exp(0.8 f1), c = exp(0.8 f2), E1s = exp(0.2 f1), E2s = exp(0.2 f2).
The E1s_i factor is constant along the softmax axis, so it cancels in the
normalization and is NEVER computed.  Per (head, j-tile) the unnormalized
attention U^T[j,i] = adj * max(d_i*c_j, 1) * E2s_j needs only TWO
DVE-class ops on the [128,1024] tile:
  1. P = (dbc * c_j) max 1     -- one 2-op tensor_scalar (4x mode), or on the
     ACT engine as R = relu(c_j*dbc - 1) (with U = (R+1)*adj split into an
     extra PE matmul against raw adj, since U = R*adj + adj).
  2. U = P * adj               -- one tensor_tensor mult (2x mode), batched
     over head PAIRS ([128, 2048] with a duplicated adjacency tile).
The E2s_j factor rides in the matmul weights [Wh*E2s | E2s]; column 64 of
the output yields the softmax denominator Z.  GPSIMD does NO elementwise
work (it shares an SBUF port with the DVE and poisons its throughput).

Finalize: ot[65,1024] PSUM -> bf16 SBUF (ACT) -> xbar DMA transpose ->
[128, 8, 65] -> batched reciprocal of Z -> eight 4x-mode tensor_scalar
muls -> single bf16 output DMA (host casts to f32).
"""

import numpy as np
import ml_dtypes

BF16 = ml_dtypes.bfloat16

B, N, D_IN, D_OUT, H = 8, 1024, 256, 64, 8
NEG_SLOPE = 0.2
P = 128                       # partitions
NJT = N // P                  # 8 j-tiles
NIT = N // P                  # 8 i-tiles
NKT = D_IN // P               # 2 k-tiles
HF = H * D_OUT                # 512
AUG = D_OUT + 1               # 65 (Wh columns + Z column)
TP = 80                       # xbar-transpose row pad (must be mult of 16)
NPAIR = H // 2                # head pairs

# ---- knobs -----------------------------------------------------------------
ACT_Y = 40        # how many of the 64 (h, jt) tiles build P on the ACT engine
XBAR_FOLD_A = False  # dma transpose row fold: False -> row r lands (p=r%128,c=r//128)


def _act_routed(h, jt):
    # bijective spread of exactly ACT_Y tiles across (h, jt)
    return ((h + H * jt) * 5) % 64 < ACT_Y


def _build_program():
    """Build the single-core SPMD Bass program."""
    import concourse.bass as bass
    import concourse.bacc as bacc
    import concourse.tile as tile
    from concourse import mybir
    from concourse.masks import make_identity

    f32 = mybir.dt.float32
    bf16 = mybir.dt.bfloat16
    AF = mybir.ActivationFunctionType
    OP = mybir.AluOpType

    nc = bacc.Bacc("TRN2", target_bir_lowering=False, debug=False,
                   enable_asserts=False, num_devices=8)

    hT = nc.dram_tensor("hT", [D_IN, N], bf16, kind="ExternalInput").ap()
    adjT = nc.dram_tensor("adjT", [N, N], bf16, kind="ExternalInput").ap()
    wrs = nc.dram_tensor("wrs", [D_IN, HF], bf16, kind="ExternalInput").ap()
    w12 = nc.dram_tensor("w12", [D_IN, 2 * H], bf16,
                         kind="ExternalInput").ap()
    out = nc.dram_tensor("out", [N, HF], bf16, kind="ExternalOutput").ap()

    with tile.TileContext(nc) as tc:
        with (
            tc.tile_pool(name="const", bufs=1) as const,
            tc.tile_pool(name="inputs", bufs=1) as inputs,
            tc.tile_pool(name="whp", bufs=1) as whp,
            tc.tile_pool(name="ecol", bufs=1) as ecolp,
            tc.tile_pool(name="ps_s", bufs=2, space="PSUM") as ps_s,
            tc.tile_pool(name="ps_ot", bufs=3, space="PSUM") as ps_ot,
            tc.tile_pool(name="bcast", bufs=2) as bcastp,
            tc.tile_pool(name="u2", bufs=2) as u2p,
            tc.tile_pool(name="u2m", bufs=2) as u2mp,
            tc.tile_pool(name="work", bufs=2) as work,
            tc.tile_pool(name="fin", bufs=2) as fin,
            tc.tile_pool(name="fin2", bufs=2) as fin2,
            tc.tile_pool(name="dram", bufs=1, space="DRAM") as dramp,
        ):
            # ---- Phase 0: constants + input loads -------------------------
            ident = const.tile([P, P], f32)
            make_identity(nc, ident)
            negone = const.tile([P, 1], f32)
            nc.vector.memset(negone, -1.0)

            ht_sb = []
            for kt in range(NKT):
                t = inputs.tile([P, N], bf16, tag=f"ht{kt}")
                nc.sync.dma_start(out=t, in_=hT[kt * P:(kt + 1) * P, :])
                ht_sb.append(t)
            wrs_sb = []
            for kt in range(NKT):
                t = inputs.tile([P, HF], bf16, tag=f"wrs{kt}")
                nc.sync.dma_start(out=t, in_=wrs[kt * P:(kt + 1) * P, :])
                wrs_sb.append(t)
            w12_sb = []
            for kt in range(NKT):
                t = inputs.tile([P, 2 * H], bf16, tag=f"w12{kt}")
                nc.sync.dma_start(out=t, in_=w12[kt * P:(kt + 1) * P, :])
                w12_sb.append(t)
            # adjacency, duplicated side by side for head-pair batching
            adj_sb = []
            for jt in range(NJT):
                t = inputs.tile([P, 2 * N], bf16, tag=f"adj{jt}")
                nc.sync.dma_start(out=t[:, 0:N],
                                  in_=adjT[jt * P:(jt + 1) * P, :])
                nc.sync.dma_start(out=t[:, N:2 * N],
                                  in_=adjT[jt * P:(jt + 1) * P, :])
                adj_sb.append(t)

            # ---- Phase 1: f vectors, their exponentials -------------------
            # ecols[it][:, h]     = E2s = exp(0.2 f2_h)   (whaug scale + Z col)
            # ecols[it][:, 8 + h] = c   = exp(0.8 f2_h)   (P-build scale)
            # drow[h, i]          = d   = exp(0.8 f1_h[i])
            drow = const.tile([H, N], bf16)
            ecols = []
            for it in range(NIT):
                ps = ps_s.tile([P, 2 * H], f32, tag='pss')
                for kt in range(NKT):
                    lhsT = ht_sb[kt][:, it * P:(it + 1) * P]
                    nc.tensor.matmul(ps, lhsT, w12_sb[kt],
                                     start=(kt == 0), stop=(kt == NKT - 1))
                ec = ecolp.tile([P, 2 * H], f32, tag=f"ecols{it}")
                nc.scalar.activation(ec[:, 0:H], ps[:, H:2 * H], AF.Exp,
                                     scale=NEG_SLOPE)
                nc.scalar.activation(ec[:, H:2 * H], ps[:, H:2 * H], AF.Exp,
                                     scale=1.0 - NEG_SLOPE)
                ecols.append(ec)
                fsb = work.tile([P, 2 * H], f32, tag="fsb")
                nc.scalar.copy(fsb, ps)
                tr = ps_s.tile([2 * H, P], f32, tag='pss')
                nc.tensor.transpose(tr, fsb, ident)
                nc.scalar.activation(drow[0:H, it * P:(it + 1) * P],
                                     tr[0:H, :], AF.Exp,
                                     scale=1.0 - NEG_SLOPE)

            # bounce drow through DRAM so broadcasts can use 0-stride reads
            drow_dram = dramp.tile([1, H * N], bf16)
            for h in range(H):
                nc.sync.dma_start(out=drow_dram[:, h * N:(h + 1) * N],
                                  in_=drow[h:h + 1, :])

            # ---- Phase 2: whaug = [Wh * E2s | E2s] ------------------------
            whaug = []
            for it in range(NIT):
                ps = ps_s.tile([P, HF], f32, tag='pss')
                for kt in range(NKT):
                    lhsT = ht_sb[kt][:, it * P:(it + 1) * P]
                    nc.tensor.matmul(ps, lhsT, wrs_sb[kt],
                                     start=(kt == 0), stop=(kt == NKT - 1))
                whs = work.tile([P, HF], bf16, tag="whs")
                nc.vector.tensor_copy(whs, ps)
                wa = whp.tile([P, H, AUG], bf16, tag=f"whaug{it}")
                for h in range(H):
                    nc.vector.tensor_scalar_mul(
                        wa[:, h, 0:D_OUT],
                        whs[:, h * D_OUT:(h + 1) * D_OUT],
                        ecols[it][:, h:h + 1])
                nc.vector.tensor_copy(wa[:, :, D_OUT], ecols[it][:, 0:H])
                whaug.append(wa)

            # full output tile: [p, c, h*64+f]; row i of the output lives at
            # (p, c) per the xbar fold (see XBAR_FOLD_A)
            out_sb = whp.tile([P, NIT, HF], bf16, tag="osb")

            # standing padded evac tiles (xbar needs rows % 16 == 0); the
            # pad rows are initialized once and never read back
            ev_tiles = []
            for k in range(2):
                evt = whp.tile([TP, N], bf16, tag=f"ev{k}")
                nc.vector.memset(evt[D_OUT:TP, :], 0.0)
                ev_tiles.append(evt)

            # ---- Phase 3: attention, one head PAIR at a time --------------
            for pr in range(NPAIR):
                h0 = 2 * pr
                db2 = bcastp.tile([P, 2 * N], bf16, tag="db2")
                nc.sync.dma_start(
                    out=db2,
                    in_=drow_dram[:, h0 * N:(h0 + 2) * N]
                        .partition_broadcast(P))

                ots = []
                nmm = []   # matmuls per accumulation group (per nh slice)
                for hh in range(2):
                    ot = ps_ot.tile([AUG, N], f32, tag="ot")
                    ots.append(ot)
                    nmm.append(NJT + sum(
                        1 for jt in range(NJT) if _act_routed(h0 + hh, jt)))
                cnt = [0, 0]

                for jt in range(NJT):
                    u2 = u2p.tile([P, 2 * N], bf16, tag="u2")
                    routed = []
                    for hh in range(2):
                        h = h0 + hh
                        ccol = ecols[jt][:, H + h:H + h + 1]
                        half = u2[:, hh * N:(hh + 1) * N]
                        dhalf = db2[:, hh * N:(hh + 1) * N]
                        r = _act_routed(h, jt)
                        routed.append(r)
                        if r:
                            # R = relu(c_j * d - 1)  (the +1 rides on an
                            # extra matmul against raw adj)
                            nc.scalar.activation(half, dhalf, AF.Relu,
                                                 bias=negone, scale=ccol)
                        else:
                            # P = max(c_j * d, 1)
                            nc.vector.tensor_scalar(
                                half, dhalf, ccol, 1.0, OP.mult, OP.max)
                    u2m = u2mp.tile([P, 2 * N], bf16, tag="u2m")
                    nc.vector.tensor_tensor(out=u2m, in0=u2, in1=adj_sb[jt],
                                            op=OP.mult)
                    for hh in range(2):
                        h = h0 + hh
                        lhsT = whaug[jt][:, h, :]
                        first = cnt[hh] == 0
                        cnt[hh] += 1
                        last = cnt[hh] == nmm[hh]
                        for nh in range(2):
                            nc.tensor.matmul(
                                ots[hh][:, nh * 512:(nh + 1) * 512], lhsT,
                                u2m[:, hh * N + nh * 512:
                                    hh * N + (nh + 1) * 512],
                                start=first, stop=last)
                        if routed[hh]:
                            first = False
                            cnt[hh] += 1
                            last = cnt[hh] == nmm[hh]
                            for nh in range(2):
                                nc.tensor.matmul(
                                    ots[hh][:, nh * 512:(nh + 1) * 512], lhsT,
                                    adj_sb[jt][:, nh * 512:(nh + 1) * 512],
                                    start=False, stop=last)

                    # ---- finalize each head of the pair -------------------
                for hh in range(2):
                    h = h0 + hh
                    ev = ev_tiles[hh]
                    nc.scalar.copy(ev[0:AUG, :], ots[hh])
                    ttile = fin2.tile([P, NIT, TP], bf16, tag="tt")
                    nc.sync.dma_start_transpose(out=ttile, in_=ev)
                    rcol = fin.tile([P, NIT], f32, tag="rcol")
                    nc.vector.reciprocal(rcol, ttile[:, :, D_OUT])
                    for c in range(NIT):
                        nc.vector.tensor_scalar_mul(
                            out_sb[:, c, h * D_OUT:(h + 1) * D_OUT],
                            ttile[:, c, 0:D_OUT], rcol[:, c:c + 1])

            # ---- Phase 4: store -------------------------------------------
            if XBAR_FOLD_A:
                # transposed row r landed at (p=r//8, c=r%8)
                oview = out.rearrange("(p c) f -> p c f", c=NIT)
            else:
                # row r landed at (p=r%128, c=r//128)
                oview = out.rearrange("(c p) f -> p c f", p=P)
            nc.sync.dma_start(out=oview, in_=out_sb)

    nc.compile()
    return nc


def _host_prep(h, adj, W, a):
    """Host-side input prep: transposes / casts / tiny einsums only."""
    a1, a2 = a[:, :D_OUT], a[:, D_OUT:]
    w1 = np.einsum("hdf,hf->hd", W, a1).astype(np.float32)   # [H, D_IN]
    w2 = np.einsum("hdf,hf->hd", W, a2).astype(np.float32)
    w12 = np.concatenate([w1.T, w2.T], axis=1).astype(BF16)  # [D_IN, 16]
    wrs = np.ascontiguousarray(
        W.transpose(1, 0, 2).reshape(D_IN, HF)).astype(BF16)
    in_maps = []
    for b in range(B):
        in_maps.append({
            "hT": np.ascontiguousarray(h[b].T).astype(BF16),
            "adjT": np.ascontiguousarray(adj[b].T).astype(BF16),
            "wrs": wrs,
            "w12": w12,
        })
    return in_maps


def kernel(h, adj, W, a):
    from concourse.bass_utils import run_bass_kernel_spmd

    in_maps = _host_prep(np.asarray(h), np.asarray(adj),
                         np.asarray(W), np.asarray(a))
    nc = _build_program()
    res = run_bass_kernel_spmd(nc, in_maps, core_ids=list(range(B)))
    out = np.stack([np.asarray(res.results[b]["out"]).astype(np.float32)
                    for b in range(B)])
    return out


# revision 14
# speedup vs baseline: 1.6805x; 1.1035x over previous
"""Multi-head graph attention layer (GAT) for Trainium2, 8-core data-parallel.

Problem: B=8, N=1024, D_IN=256, D_OUT=64, H=8, LeakyReLU slope 0.2.
  Wh = einsum('bnd,hdf->bhnf', h, W)
  f1 = Wh @ a1, f2 = Wh @ a2              (per head)
  e  = leaky_relu(f1[:,None] + f2[None,:])
  att = softmax(where(adj==0, -inf, e))
  out = att @ Wh  -> concat heads [B, N, H*F]

Sharding: one batch element per NeuronCore (B=8 across 8 cores).

Algebra: with x = f1_i + f2_j,
  exp(leaky_relu(x)) = max(exp(x), exp(0.2 x))
                     = E1s_i * E2s_j * max(d_i * c_j, 1)
where d = exp(0.8 f1), c = exp(0.8 f2), E1s = exp(0.2 f1), E2s = exp(0.2 f2).
The E1s_i factor is constant along the softmax axis, so it cancels in the
normalization and is NEVER computed.  Per (head, j-tile) the unnormalized
attention U^T[j,i] = adj * max(d_i*c_j, 1) * E2s_j needs only TWO
DVE-class ops on the [128,1024] tile:
  1. P = (dbc * c_j) max 1     -- one 2-op tensor_scalar (4x mode), or on the
     ACT engine as R = relu(c_j*dbc - 1) (with U = (R+1)*adj split into an
     extra PE matmul against raw adj, since U = R*adj + adj).
  2. U = P * adj               -- one tensor_tensor mult (2x mode), batched
     over head PAIRS ([128, 2048] with a duplicated adjacency tile).
The E2s_j factor rides in the matmul weights [Wh*E2s | E2s]; column 64 of
the output yields the softmax denominator Z.  GPSIMD does NO elementwise
work (it shares an SBUF port with the DVE and poisons its throughput).

Finalize: ot[65,1024] PSUM -> bf16 SBUF (ACT) -> xbar DMA transpose ->
[128, 8, 65] -> batched reciprocal of Z -> eight 4x-mode tensor_scalar
muls -> single bf16 output DMA (host casts to f32).
"""

import numpy as np
import ml_dtypes

BF16 = ml_dtypes.bfloat16

B, N, D_IN, D_OUT, H = 8, 1024, 256, 64, 8
NEG_SLOPE = 0.2
P = 128                       # partitions
NJT = N // P                  # 8 j-tiles
NIT = N // P                  # 8 i-tiles
NKT = D_IN // P               # 2 k-tiles
HF = H * D_OUT                # 512
AUG = D_OUT + 1               # 65 (Wh columns + Z column)
TP = 80                       # xbar-transpose row pad (must be mult of 16)
NPAIR = H // 2                # head pairs

# ---- knobs -----------------------------------------------------------------
ACT_Y = 32        # how many of the 64 (h, jt) tiles build P on the ACT engine
XBAR_FOLD_A = False  # dma transpose row fold: False -> row r lands (p=r%128,c=r//128)


def _act_routed(h, jt):
    # bijective spread of exactly ACT_Y tiles across (h, jt)
    return ((h + H * jt) * 5) % 64 < ACT_Y


def _build_program():
    """Build the single-core SPMD Bass program."""
    import concourse.bass as bass
    import concourse.bacc as bacc
    import concourse.tile as tile
    from concourse import mybir
    from concourse.masks import make_identity

    f32 = mybir.dt.float32
    bf16 = mybir.dt.bfloat16
    AF = mybir.ActivationFunctionType
    OP = mybir.AluOpType

    nc = bacc.Bacc("TRN2", target_bir_lowering=False, debug=False,
                   enable_asserts=False, num_devices=8)

    hT = nc.dram_tensor("hT", [D_IN, N], bf16, kind="ExternalInput").ap()
    adjT = nc.dram_tensor("adjT", [N, N], bf16, kind="ExternalInput").ap()
    wrs = nc.dram_tensor("wrs", [D_IN, HF], bf16, kind="ExternalInput").ap()
    w12 = nc.dram_tensor("w12", [D_IN, 2 * H], bf16,
                         kind="ExternalInput").ap()
    out = nc.dram_tensor("out", [N, HF], bf16, kind="ExternalOutput").ap()

    with tile.TileContext(nc) as tc:
        with (
            tc.tile_pool(name="const", bufs=1) as const,
            tc.tile_pool(name="inputs", bufs=1) as inputs,
            tc.tile_pool(name="whp", bufs=1) as whp,
            tc.tile_pool(name="ecol", bufs=1) as ecolp,
            tc.tile_pool(name="psp", bufs=4, space="PSUM") as psp,
            tc.tile_pool(name="bcast", bufs=2) as bcastp,
            tc.tile_pool(name="u2", bufs=3) as u2p,
            tc.tile_pool(name="u2m", bufs=3) as u2mp,
            tc.tile_pool(name="work", bufs=2) as work,
            tc.tile_pool(name="fin", bufs=2) as fin,
            tc.tile_pool(name="fin2", bufs=2) as fin2,
            tc.tile_pool(name="dram", bufs=1, space="DRAM") as dramp,
        ):
            # ---- Phase 0: constants + input loads -------------------------
            ident = const.tile([P, P], f32)
            make_identity(nc, ident)
            negone = const.tile([P, 1], f32)
            nc.vector.memset(negone, -1.0)

            ht_sb = []
            for kt in range(NKT):
                t = inputs.tile([P, N], bf16, tag=f"ht{kt}")
                nc.sync.dma_start(out=t, in_=hT[kt * P:(kt + 1) * P, :])
                ht_sb.append(t)
            wrs_sb = []
            for kt in range(NKT):
                t = inputs.tile([P, HF], bf16, tag=f"wrs{kt}")
                nc.sync.dma_start(out=t, in_=wrs[kt * P:(kt + 1) * P, :])
                wrs_sb.append(t)
            w12_sb = []
            for kt in range(NKT):
                t = inputs.tile([P, 2 * H], bf16, tag=f"w12{kt}")
                nc.sync.dma_start(out=t, in_=w12[kt * P:(kt + 1) * P, :])
                w12_sb.append(t)
            # adjacency, duplicated side by side for head-pair batching
            adj_sb = []
            for jt in range(NJT):
                t = inputs.tile([P, 2 * N], bf16, tag=f"adj{jt}")
                nc.sync.dma_start(out=t[:, 0:N],
                                  in_=adjT[jt * P:(jt + 1) * P, :])
                nc.sync.dma_start(out=t[:, N:2 * N],
                                  in_=adjT[jt * P:(jt + 1) * P, :])
                adj_sb.append(t)

            # ---- Phase 1+2 (interleaved): f exponentials + whaug ----------
            # ecols[it][:, h]     = E2s = exp(0.2 f2_h)   (whaug scale + Z col)
            # ecols[it][:, 8 + h] = c   = exp(0.8 f2_h)   (P-build scale)
            # drow[h, i]          = d   = exp(0.8 f1_h[i])
            # whaug[it]           = [Wh * E2s | E2s]
            drow = const.tile([H, N], bf16)
            ecols, whaug = [], []
            for it in range(NIT):
                ps = psp.tile([P, 2 * H], f32, tag='ps')
                for kt in range(NKT):
                    lhsT = ht_sb[kt][:, it * P:(it + 1) * P]
                    nc.tensor.matmul(ps, lhsT, w12_sb[kt],
                                     start=(kt == 0), stop=(kt == NKT - 1))
                ec = ecolp.tile([P, 2 * H], f32, tag=f"ecols{it}")
                nc.scalar.activation(ec[:, 0:H], ps[:, H:2 * H], AF.Exp,
                                     scale=NEG_SLOPE)
                nc.scalar.activation(ec[:, H:2 * H], ps[:, H:2 * H], AF.Exp,
                                     scale=1.0 - NEG_SLOPE)
                ecols.append(ec)
                fsb = work.tile([P, 2 * H], f32, tag="fsb")
                nc.scalar.copy(fsb, ps)
                tr = psp.tile([2 * H, P], f32, tag='ps')
                nc.tensor.transpose(tr, fsb, ident)
                nc.scalar.activation(drow[0:H, it * P:(it + 1) * P],
                                     tr[0:H, :], AF.Exp,
                                     scale=1.0 - NEG_SLOPE)

                wh = psp.tile([P, HF], f32, tag='ps')
                for kt in range(NKT):
                    lhsT = ht_sb[kt][:, it * P:(it + 1) * P]
                    nc.tensor.matmul(wh, lhsT, wrs_sb[kt],
                                     start=(kt == 0), stop=(kt == NKT - 1))
                wa = whp.tile([P, H, AUG], bf16, tag=f"whaug{it}")
                nc.vector.tensor_tensor(
                    out=wa[:, :, 0:D_OUT],
                    in0=wh.rearrange('p (h f) -> p h f', f=D_OUT),
                    in1=ec[:, 0:H].unsqueeze(2).broadcast_to([P, H, D_OUT]),
                    op=OP.mult)
                nc.vector.tensor_copy(wa[:, :, D_OUT], ec[:, 0:H])
                whaug.append(wa)

            # bounce drow through DRAM so broadcasts can use 0-stride reads
            drow_dram = dramp.tile([1, H * N], bf16)
            for h in range(H):
                nc.sync.dma_start(out=drow_dram[:, h * N:(h + 1) * N],
                                  in_=drow[h:h + 1, :])

            # full output tile: [p, c, h*64+f]; row i of the output lives at
            # (p, c) per the xbar fold (see XBAR_FOLD_A)
            out_sb = whp.tile([P, NIT, HF], bf16, tag="osb")

            # standing padded evac tiles (xbar needs rows % 16 == 0); the
            # pad rows are initialized once and never read back
            ev_tiles = []
            for k in range(2):
                evt = whp.tile([TP, N], bf16, tag=f"ev{k}")
                nc.vector.memset(evt[D_OUT:TP, :], 0.0)
                ev_tiles.append(evt)

            # ---- Phase 3: attention, one head PAIR at a time --------------
            for pr in range(NPAIR):
                h0 = 2 * pr
                db2 = bcastp.tile([P, 2 * N], bf16, tag="db2")
                nc.sync.dma_start(
                    out=db2,
                    in_=drow_dram[:, h0 * N:(h0 + 2) * N]
                        .partition_broadcast(P))

                ots = []
                nmm = []   # matmuls per accumulation group (per nh slice)
                for hh in range(2):
                    ot = psp.tile([AUG, N], f32, tag="ps")
                    ots.append(ot)
                    nmm.append(NJT + sum(
                        1 for jt in range(NJT) if _act_routed(h0 + hh, jt)))
                cnt = [0, 0]

                for jt in range(NJT):
                    u2 = u2p.tile([P, 2 * N], bf16, tag="u2")
                    routed = []
                    for hh in range(2):
                        h = h0 + hh
                        ccol = ecols[jt][:, H + h:H + h + 1]
                        half = u2[:, hh * N:(hh + 1) * N]
                        dhalf = db2[:, hh * N:(hh + 1) * N]
                        r = _act_routed(h, jt)
                        routed.append(r)
                        if r:
                            # R = relu(c_j * d - 1)  (the +1 rides on an
                            # extra matmul against raw adj)
                            nc.scalar.activation(half, dhalf, AF.Relu,
                                                 bias=negone, scale=ccol)
                        else:
                            # P = max(c_j * d, 1)
                            nc.vector.tensor_scalar(
                                half, dhalf, ccol, 1.0, OP.mult, OP.max)
                    u2m = u2mp.tile([P, 2 * N], bf16, tag="u2m")
                    nc.vector.tensor_tensor(out=u2m, in0=u2, in1=adj_sb[jt],
                                            op=OP.mult)
                    for hh in range(2):
                        h = h0 + hh
                        lhsT = whaug[jt][:, h, :]
                        first = cnt[hh] == 0
                        cnt[hh] += 1
                        last = cnt[hh] == nmm[hh]
                        for nh in range(2):
                            nc.tensor.matmul(
                                ots[hh][:, nh * 512:(nh + 1) * 512], lhsT,
                                u2m[:, hh * N + nh * 512:
                                    hh * N + (nh + 1) * 512],
                                start=first, stop=last)
                        if routed[hh]:
                            first = False
                            cnt[hh] += 1
                            last = cnt[hh] == nmm[hh]
                            for nh in range(2):
                                nc.tensor.matmul(
                                    ots[hh][:, nh * 512:(nh + 1) * 512], lhsT,
                                    adj_sb[jt][:, nh * 512:(nh + 1) * 512],
                                    start=False, stop=last)

                    # ---- finalize each head of the pair -------------------
                for hh in range(2):
                    h = h0 + hh
                    ev = ev_tiles[hh]
                    nc.scalar.copy(ev[0:AUG, :], ots[hh])
                    ttile = fin2.tile([P, NIT, TP], bf16, tag="tt")
                    nc.sync.dma_start_transpose(out=ttile, in_=ev)
                    rcol = fin.tile([P, NIT], f32, tag="rcol")
                    nc.vector.reciprocal(rcol, ttile[:, :, D_OUT])
                    nc.vector.tensor_tensor(
                        out=out_sb[:, :, h * D_OUT:(h + 1) * D_OUT],
                        in0=ttile[:, :, 0:D_OUT],
                        in1=rcol.unsqueeze(2).broadcast_to([P, NIT, D_OUT]),
                        op=OP.mult)

            # ---- Phase 4: store -------------------------------------------
            if XBAR_FOLD_A:
                # transposed row r landed at (p=r//8, c=r%8)
                oview = out.rearrange("(p c) f -> p c f", c=NIT)
            else:
                # row r landed at (p=r%128, c=r//128)
                oview = out.rearrange("(c p) f -> p c f", p=P)
            nc.sync.dma_start(out=oview, in_=out_sb)

    nc.compile()
    return nc


def _host_prep(h, adj, W, a):
    """Host-side input prep: transposes / casts / tiny einsums only."""
    a1, a2 = a[:, :D_OUT], a[:, D_OUT:]
    w1 = np.einsum("hdf,hf->hd", W, a1).astype(np.float32)   # [H, D_IN]
    w2 = np.einsum("hdf,hf->hd", W, a2).astype(np.float32)
    w12 = np.concatenate([w1.T, w2.T], axis=1).astype(BF16)  # [D_IN, 16]
    wrs = np.ascontiguousarray(
        W.transpose(1, 0, 2).reshape(D_IN, HF)).astype(BF16)
    in_maps = []
    for b in range(B):
        in_maps.append({
            "hT": np.ascontiguousarray(h[b].T).astype(BF16),
            "adjT": np.ascontiguousarray(adj[b].T).astype(BF16),
            "wrs": wrs,
            "w12": w12,
        })
    return in_maps


def kernel(h, adj, W, a):
    from concourse.bass_utils import run_bass_kernel_spmd

    in_maps = _host_prep(np.asarray(h), np.asarray(adj),
                         np.asarray(W), np.asarray(a))
    nc = _build_program()
    res = run_bass_kernel_spmd(nc, in_maps, core_ids=list(range(B)))
    out = np.stack([np.asarray(res.results[b]["out"]).astype(np.float32)
                    for b in range(B)])
    return out


# revision 18
# speedup vs baseline: 1.8521x; 1.1021x over previous
"""Multi-head graph attention layer (GAT) for Trainium2, 8-core data-parallel.

Problem: B=8, N=1024, D_IN=256, D_OUT=64, H=8, LeakyReLU slope 0.2.
  Wh = einsum('bnd,hdf->bhnf', h, W)
  f1 = Wh @ a1, f2 = Wh @ a2              (per head)
  e  = leaky_relu(f1[:,None] + f2[None,:])
  att = softmax(where(adj==0, -inf, e))
  out = att @ Wh  -> concat heads [B, N, H*F]

Sharding: one batch element per NeuronCore (B=8 across 8 cores).

Algebra: with x = f1_i + f2_j,
  exp(leaky_relu(x)) = max(exp(x), exp(0.2 x))
                     = E1s_i * E2s_j * max(d_i * c_j, 1)
where d = exp(0.8 f1), c = exp(0.8 f2), E1s = exp(0.2 f1), E2s = exp(0.2 f2).
The E1s_i factor is constant along the softmax axis, so it cancels in the
normalization and is NEVER computed.  Per (head, j-tile) the unnormalized
attention U^T[j,i] = adj * max(d_i*c_j, 1) * E2s_j needs only TWO
DVE-class ops on the [128,1024] tile:
  1. P = (dbc * c_j) max 1     -- one 2-op tensor_scalar (4x mode), or on the
     ACT engine as R = relu(c_j*dbc - 1) (with U = (R+1)*adj split into an
     extra PE matmul against raw adj, since U = R*adj + adj).
  2. U = P * adj               -- one tensor_tensor mult (2x mode), batched
     over head PAIRS ([128, 2048] with a duplicated adjacency tile).
The E2s_j factor rides in the matmul weights [Wh*E2s | E2s]; column 64 of
the output yields the softmax denominator Z.  GPSIMD does NO elementwise
work (it shares an SBUF port with the DVE and poisons its throughput).

Finalize: ot[65,1024] PSUM -> bf16 SBUF (ACT) -> xbar DMA transpose ->
[128, 8, 65] -> batched reciprocal of Z -> eight 4x-mode tensor_scalar
muls -> single bf16 output DMA (host casts to f32).
"""

import numpy as np
import ml_dtypes

BF16 = ml_dtypes.bfloat16

B, N, D_IN, D_OUT, H = 8, 1024, 256, 64, 8
NEG_SLOPE = 0.2
P = 128                       # partitions
NJT = N // P                  # 8 j-tiles
NIT = N // P                  # 8 i-tiles
NKT = D_IN // P               # 2 k-tiles
HF = H * D_OUT                # 512
AUG = D_OUT + 1               # 65 (Wh columns + Z column)
TP = 80                       # xbar-transpose row pad (must be mult of 16)
NPAIR = H // 2                # head pairs

# ---- knobs -----------------------------------------------------------------
ACT_Y = 36        # how many of the 64 (h, jt) tiles build P on the ACT engine
XBAR_FOLD_A = False  # dma transpose row fold: False -> row r lands (p=r%128,c=r//128)


def _act_routed(h, jt):
    # bijective spread of exactly ACT_Y tiles across (h, jt)
    return ((h + H * jt) * 5) % 64 < ACT_Y


def _build_program():
    """Build the single-core SPMD Bass program."""
    import concourse.bass as bass
    import concourse.bacc as bacc
    import concourse.tile as tile
    from concourse import mybir
    from concourse.masks import make_identity

    f32 = mybir.dt.float32
    bf16 = mybir.dt.bfloat16
    AF = mybir.ActivationFunctionType
    OP = mybir.AluOpType

    nc = bacc.Bacc("TRN2", target_bir_lowering=False, debug=False,
                   enable_asserts=False, num_devices=8)

    hT = nc.dram_tensor("hT", [D_IN, N], bf16, kind="ExternalInput").ap()
    adjT = nc.dram_tensor("adjT", [N, N], bf16, kind="ExternalInput").ap()
    wrs = nc.dram_tensor("wrs", [D_IN, HF], bf16, kind="ExternalInput").ap()
    w12 = nc.dram_tensor("w12", [D_IN, 2 * H], bf16,
                         kind="ExternalInput").ap()
    out = nc.dram_tensor("out", [N, HF], bf16, kind="ExternalOutput").ap()

    with tile.TileContext(nc) as tc:
        with (
            tc.tile_pool(name="const", bufs=1) as const,
            tc.tile_pool(name="inputs", bufs=1) as inputs,
            tc.tile_pool(name="whp", bufs=1) as whp,
            tc.tile_pool(name="ecol", bufs=1) as ecolp,
            tc.tile_pool(name="psp", bufs=4, space="PSUM") as psp,
            tc.tile_pool(name="bcast", bufs=2) as bcastp,
            tc.tile_pool(name="u2", bufs=4) as u2p,
            tc.tile_pool(name="u2m", bufs=4) as u2mp,
            tc.tile_pool(name="work", bufs=2) as work,
            tc.tile_pool(name="fin", bufs=2) as fin,
            tc.tile_pool(name="fin2", bufs=2) as fin2,
            tc.tile_pool(name="dram", bufs=1, space="DRAM") as dramp,
        ):
            # ---- Phase 0: constants + input loads -------------------------
            ident = const.tile([P, P], f32)
            make_identity(nc, ident)
            negone = const.tile([P, 1], f32)
            nc.vector.memset(negone, -1.0)

            ht_sb = []
            for kt in range(NKT):
                t = inputs.tile([P, N], bf16, tag=f"ht{kt}")
                nc.sync.dma_start(out=t, in_=hT[kt * P:(kt + 1) * P, :])
                ht_sb.append(t)
            wrs_sb = []
            for kt in range(NKT):
                t = inputs.tile([P, HF], bf16, tag=f"wrs{kt}")
                nc.sync.dma_start(out=t, in_=wrs[kt * P:(kt + 1) * P, :])
                wrs_sb.append(t)
            w12_sb = []
            for kt in range(NKT):
                t = inputs.tile([P, 2 * H], bf16, tag=f"w12{kt}")
                nc.sync.dma_start(out=t, in_=w12[kt * P:(kt + 1) * P, :])
                w12_sb.append(t)
            # adjacency, duplicated side by side for head-pair batching
            adj_sb = []
            for jt in range(NJT):
                t = inputs.tile([P, 2 * N], bf16, tag=f"adj{jt}")
                nc.sync.dma_start(out=t[:, 0:N],
                                  in_=adjT[jt * P:(jt + 1) * P, :])
                nc.sync.dma_start(out=t[:, N:2 * N],
                                  in_=adjT[jt * P:(jt + 1) * P, :])
                adj_sb.append(t)

            # ---- Phase 1+2, dependency-ordered waves ----------------------
            # ecols[it][:, h]     = E2s = exp(0.2 f2_h)   (whaug scale + Z col)
            # ecols[it][:, 8 + h] = c   = exp(0.8 f2_h)   (P-build scale)
            # drow[h, i]          = d   = exp(0.8 f1_h[i])
            # whaug[it]           = [Wh * E2s | E2s]
            drow = const.tile([H, N], bf16)

            # wave A: all f matmuls, evacuated to SBUF immediately
            fsbs = []
            for it in range(NIT):
                ps = psp.tile([P, 2 * H], f32, tag='ps')
                for kt in range(NKT):
                    lhsT = ht_sb[kt][:, it * P:(it + 1) * P]
                    nc.tensor.matmul(ps, lhsT, w12_sb[kt],
                                     start=(kt == 0), stop=(kt == NKT - 1))
                fsb = ecolp.tile([P, 2 * H], f32, tag=f"fsb{it}")
                nc.scalar.copy(fsb, ps)
                fsbs.append(fsb)

            # wave B: transpose f1, exponentiate into drow (gates phase 3)
            for it in range(NIT):
                tr = psp.tile([2 * H, P], f32, tag='ps')
                nc.tensor.transpose(tr, fsbs[it], ident)
                nc.scalar.activation(drow[0:H, it * P:(it + 1) * P],
                                     tr[0:H, :], AF.Exp,
                                     scale=1.0 - NEG_SLOPE)

            # single-DMA bounce of drow through DRAM for 0-stride broadcasts
            drow_dram = dramp.tile([H, N], bf16)
            nc.sync.dma_start(out=drow_dram, in_=drow)
            drow_flat = drow_dram.rearrange('a b -> (a b)').unsqueeze(0)

            # wave C: per-j-tile exponentials (from SBUF copies)
            ecols = []
            for it in range(NIT):
                ec = ecolp.tile([P, 2 * H], f32, tag=f"ecols{it}")
                nc.scalar.activation(ec[:, 0:H], fsbs[it][:, H:2 * H],
                                     AF.Exp, scale=NEG_SLOPE)
                nc.scalar.activation(ec[:, H:2 * H], fsbs[it][:, H:2 * H],
                                     AF.Exp, scale=1.0 - NEG_SLOPE)
                ecols.append(ec)

            # wave D: Wh matmuls + whaug (PE/DVE only, overlaps B/C)
            whaug = []
            for it in range(NIT):
                wh = psp.tile([P, HF], f32, tag='ps')
                for kt in range(NKT):
                    lhsT = ht_sb[kt][:, it * P:(it + 1) * P]
                    nc.tensor.matmul(wh, lhsT, wrs_sb[kt],
                                     start=(kt == 0), stop=(kt == NKT - 1))
                wa = whp.tile([P, H, AUG], bf16, tag=f"whaug{it}")
                nc.vector.tensor_tensor(
                    out=wa[:, :, 0:D_OUT],
                    in0=wh.rearrange('p (h f) -> p h f', f=D_OUT),
                    in1=ecols[it][:, 0:H].unsqueeze(2)
                        .broadcast_to([P, H, D_OUT]),
                    op=OP.mult)
                nc.vector.tensor_copy(wa[:, :, D_OUT], ecols[it][:, 0:H])
                whaug.append(wa)

            # full output tile: [p, c, h*64+f]; row i of the output lives at
            # (p, c) per the xbar fold (see XBAR_FOLD_A)
            out_sb = whp.tile([P, NIT, HF], bf16, tag="osb")

            # standing padded evac tiles (xbar needs rows % 16 == 0); the
            # pad rows are initialized once and never read back
            ev_tiles = []
            for k in range(2):
                evt = whp.tile([TP, N], bf16, tag=f"ev{k}")
                nc.vector.memset(evt[D_OUT:TP, :], 0.0)
                ev_tiles.append(evt)

            # ---- Phase 3: attention, one head PAIR at a time --------------
            for pr in range(NPAIR):
                h0 = 2 * pr
                db2 = bcastp.tile([P, 2 * N], bf16, tag="db2")
                nc.sync.dma_start(
                    out=db2,
                    in_=drow_flat[:, h0 * N:(h0 + 2) * N]
                        .partition_broadcast(P))

                ots = []
                nmm = []   # matmuls per accumulation group (per nh slice)
                for hh in range(2):
                    ot = psp.tile([AUG, N], f32, tag="ps")
                    ots.append(ot)
                    nmm.append(NJT + sum(
                        1 for jt in range(NJT) if _act_routed(h0 + hh, jt)))
                cnt = [0, 0]

                for jt in range(NJT):
                    u2 = u2p.tile([P, 2 * N], bf16, tag="u2")
                    routed = []
                    for hh in range(2):
                        h = h0 + hh
                        ccol = ecols[jt][:, H + h:H + h + 1]
                        half = u2[:, hh * N:(hh + 1) * N]
                        dhalf = db2[:, hh * N:(hh + 1) * N]
                        r = _act_routed(h, jt)
                        routed.append(r)
                        if r:
                            # R = relu(c_j * d - 1)  (the +1 rides on an
                            # extra matmul against raw adj)
                            nc.scalar.activation(half, dhalf, AF.Relu,
                                                 bias=negone, scale=ccol)
                        else:
                            # P = max(c_j * d, 1)
                            nc.vector.tensor_scalar(
                                half, dhalf, ccol, 1.0, OP.mult, OP.max)
                    u2m = u2mp.tile([P, 2 * N], bf16, tag="u2m")
                    nc.vector.tensor_tensor(out=u2m, in0=u2, in1=adj_sb[jt],
                                            op=OP.mult)
                    for hh in range(2):
                        h = h0 + hh
                        lhsT = whaug[jt][:, h, :]
                        first = cnt[hh] == 0
                        cnt[hh] += 1
                        last = cnt[hh] == nmm[hh]
                        for nh in range(2):
                            nc.tensor.matmul(
                                ots[hh][:, nh * 512:(nh + 1) * 512], lhsT,
                                u2m[:, hh * N + nh * 512:
                                    hh * N + (nh + 1) * 512],
                                start=first, stop=last)
                        if routed[hh]:
                            first = False
                            cnt[hh] += 1
                            last = cnt[hh] == nmm[hh]
                            for nh in range(2):
                                nc.tensor.matmul(
                                    ots[hh][:, nh * 512:(nh + 1) * 512], lhsT,
                                    adj_sb[jt][:, nh * 512:(nh + 1) * 512],
                                    start=False, stop=last)

                    # ---- finalize each head of the pair -------------------
                for hh in range(2):
                    h = h0 + hh
                    ev = ev_tiles[hh]
                    nc.scalar.copy(ev[0:AUG, :], ots[hh])
                    ttile = fin2.tile([P, NIT, TP], bf16, tag="tt")
                    nc.sync.dma_start_transpose(out=ttile, in_=ev)
                    rcol = fin.tile([P, NIT], f32, tag="rcol")
                    nc.vector.reciprocal(rcol, ttile[:, :, D_OUT])
                    nc.vector.tensor_tensor(
                        out=out_sb[:, :, h * D_OUT:(h + 1) * D_OUT],
                        in0=ttile[:, :, 0:D_OUT],
                        in1=rcol.unsqueeze(2).broadcast_to([P, NIT, D_OUT]),
                        op=OP.mult)

            # ---- Phase 4: store -------------------------------------------
            if XBAR_FOLD_A:
                # transposed row r landed at (p=r//8, c=r%8)
                oview = out.rearrange("(p c) f -> p c f", c=NIT)
            else:
                # row r landed at (p=r%128, c=r//128)
                oview = out.rearrange("(c p) f -> p c f", p=P)
            nc.sync.dma_start(out=oview, in_=out_sb)

    nc.compile()
    return nc


def _host_prep(h, adj, W, a):
    """Host-side input prep: transposes / casts / tiny einsums only."""
    a1, a2 = a[:, :D_OUT], a[:, D_OUT:]
    w1 = np.einsum("hdf,hf->hd", W, a1).astype(np.float32)   # [H, D_IN]
    w2 = np.einsum("hdf,hf->hd", W, a2).astype(np.float32)
    w12 = np.concatenate([w1.T, w2.T], axis=1).astype(BF16)  # [D_IN, 16]
    wrs = np.ascontiguousarray(
        W.transpose(1, 0, 2).reshape(D_IN, HF)).astype(BF16)
    in_maps = []
    for b in range(B):
        in_maps.append({
            "hT": np.ascontiguousarray(h[b].T).astype(BF16),
            "adjT": np.ascontiguousarray(adj[b].T).astype(BF16),
            "wrs": wrs,
            "w12": w12,
        })
    return in_maps


def kernel(h, adj, W, a):
    from concourse.bass_utils import run_bass_kernel_spmd

    in_maps = _host_prep(np.asarray(h), np.asarray(adj),
                         np.asarray(W), np.asarray(a))
    nc = _build_program()
    res = run_bass_kernel_spmd(nc, in_maps, core_ids=list(range(B)))
    out = np.stack([np.asarray(res.results[b]["out"]).astype(np.float32)
                    for b in range(B)])
    return out
